# revision 48
# baseline (speedup 1.0000x reference)
"""Trainium2 Bass kernel for nn_Attention_73254962200646 (Winograd version).

Reference computation (per batch element b, all shapes hardcoded):
  qkv = conv3x3(x, W_qkv, pad=1)            x:[8,512,32,32], W_qkv:[1536,512,3,3]
  q,k,v -> [g=8 heads, n=1024, d=64]
  attn  = (q @ k^T) / (|q| |k| + eps)       cosine-similarity attention
  out   = attn @ v -> [512,32,32]
  out   = conv1x1(out, W_out); BatchNorm2d (batch stats); ReLU

Distribution: data-parallel over batch B=8 across the 8 NeuronCores (one
image per core). All compute core-local in bf16 (fp32 PSUM accumulation);
the only collective is an 8KB AllGather of the BatchNorm partial sums
(each core then reduces the 8 copies locally -- the AllGather floor is
~3x lower than AllReduce's).

The conv3x3 uses Winograd F(2x2,3x3): weights are transformed to U = G g G^T
on the host; the input transform V = B^T d B (all +-1 coefficients) runs
split across the vector engine (input-channel blocks 0-2) and gpsimd
(block 3); the per-position products M_p = sum_ci U_p V_p are 16x4 matmuls
of [128,128]x[128,256] per output-channel block; M positions are
accumulated pairwise into [128,512] PSUM banks and evacuated with one
scalar copy per pair; the inverse transform Y = A^T M A runs on the
vector engine.

Attention runs as quadrant-packed matmuls: the qk product for a head pair
uses all four 64x64 PE quadrants concurrently (rows split by head d=64,
columns split by j-pixel 64-blocks), and the attn@v product uses two
64-column tiles -- no duty-filler matmuls are needed because the stream
stays dense enough to hold the HAM activity window.
"""

import numpy as np
import ml_dtypes

import concourse.tile as tile
import concourse.mybir as mybir
from concourse import bacc, bass_utils

BF = ml_dtypes.bfloat16
SMOOTH = 1e-4
BN_EPS = 1e-5
NCORES = 8

_NC = None
LAST_RESULT = None


def _build():
    f32 = mybir.dt.float32
    bf = mybir.dt.bfloat16
    AF = mybir.ActivationFunctionType
    ALU = mybir.AluOpType

    nc = bacc.Bacc("TRN2", target_bir_lowering=False, debug=False,
                   num_devices=NCORES)
    xin = nc.dram_tensor("xpad", [4, 128, 34, 34], bf, kind="ExternalInput").ap()
    wu = nc.dram_tensor("wu", [12, 4, 128, 4, 4, 128], bf,
                        kind="ExternalInput").ap()
    wo = nc.dram_tensor("wo", [4, 128, 512], bf, kind="ExternalInput").ap()
    gb = nc.dram_tensor("gb", [128, 8], f32, kind="ExternalInput").ap()
    ones2 = nc.dram_tensor("ones2", [128, 2], bf, kind="ExternalInput").ap()
    sel2 = nc.dram_tensor("sel2", [2, 128], bf, kind="ExternalInput").ap()
    ident = nc.dram_tensor("ident", [128, 128], bf, kind="ExternalInput").ap()
    out = nc.dram_tensor("out", [512, 1024], f32, kind="ExternalOutput").ap()

    with tile.TileContext(nc) as tc:
        with tc.tile_pool(name="sb", bufs=1) as sb, \
             tc.tile_pool(name="tp", bufs=2) as tp, \
             tc.tile_pool(name="ps", bufs=6, space="PSUM") as ps, \
             tc.tile_pool(name="dram", bufs=1, space="DRAM") as dram:

            xps = [sb.tile([128, 34, 34], bf, tag=f"xp{cb}", name=f"xp{cb}")
                   for cb in range(4)]
            vt = [sb.tile([128, 16, 256], bf, tag=f"vt{cb}", name=f"vt{cb}")
                  for cb in range(4)]
            identt = sb.tile([128, 128], bf, tag="identt")
            zerot = sb.tile([128, 128], bf, tag="zerot")
            wot = sb.tile([128, 4, 512], bf, tag="wot")
            gbt = sb.tile([128, 8], f32, tag="gbt")
            ones2t = sb.tile([128, 2], bf, tag="ones2t")
            sel2t = sb.tile([2, 128], bf, tag="sel2t")
            qhat = sb.tile([128, 4, 1024], bf, tag="qhat")
            khat = sb.tile([128, 4, 1024], bf, tag="khat")
            vT = sb.tile([128, 8, 512], bf, tag="vT")
            att = sb.tile([128, 4, 1024], bf, tag="att")
            yt = sb.tile([128, 4, 1024], f32, tag="yt")
            part = sb.tile([128, 16], f32, tag="part")
            gt = sb.tile([128, 8, 16], f32, tag="gt")
            t4 = sb.tile([128, 4, 16], f32, tag="t4")
            t2 = sb.tile([128, 2, 16], f32, tag="t2")
            stats = sb.tile([128, 16], f32, tag="stats")
            epst = sb.tile([128, 1], f32, tag="epst")
            smt = sb.tile([2, 1], f32, tag="smt")

            # startup DMAs: sync queue is reserved for the weight stream;
            # x tiles split across the scalar and gpsimd queues (the gpsimd
            # issues land at its queue head, before its transform ops).
            # identt goes first: the HAM warmup burst needs it immediately.
            nc.scalar.dma_start(identt[:], ident)
            nc.gpsimd.dma_start(xps[1][:], xin[1])
            nc.scalar.dma_start(xps[0][:], xin[0])
            nc.gpsimd.dma_start(xps[3][:], xin[3])
            nc.scalar.dma_start(xps[2][:], xin[2])
            nc.scalar.dma_start(ones2t[:], ones2)
            nc.scalar.dma_start(sel2t[:], sel2)
            nc.vector.memset(epst[:], BN_EPS)
            nc.vector.memset(smt[:], SMOOTH)
            nc.vector.memset(zerot[:], 0.0)

            # HAM warmup: ~4us of back-to-back matmuls flips the PE clock
            # gate from its default K=4/8 to 8/8 before the Winograd stream
            # begins; without it the free-running activity window never sees
            # a full busy period during the DMA-paced startup and the first
            # ~40us run at half clock.
            wps_warm = ps.tile([128, 128], f32, tag="patt", bufs=4,
                               name="wps_warm")
            for i in range(80):
                nc.tensor.matmul(wps_warm[:], identt[:], identt[:],
                                 start=True, stop=True,
                                 skip_group_check=True)

            def emit_duty_early(n=2):
                """identt-based filler for the startup window, before vt
                exists: keeps the HAM window busy while the V transform
                paces the first Winograd blocks."""
                dm = ps.tile([128, 128], f32, tag="patt", bufs=4,
                             name=f"dme{dumc_e[0]}")
                dumc_e[0] += 1
                for i in range(n):
                    nc.tensor.matmul(dm[:], identt[:], identt[:],
                                     start=(i == 0), stop=(i == n - 1),
                                     skip_group_check=True)

            dumc_e = [0]

            def emit_v_transform():
                """V = B^T d B, k-major so vt[:, p] lands in the order the
                matmuls consume (p = 4k + c). Split by input-channel block:
                cb 0-2 on the vector engine, cb 3 on gpsimd -- gpsimd's
                2-input throughput is ~2.5x worse, so it gets a quarter of
                the work and the two engines finish together."""
                for k in range(4):
                    tks = []
                    for cb in range(4):
                        eng = nc.gpsimd if cb == 3 else nc.vector
                        tk = tp.tile([128, 16, 34], bf, tag="tkw", bufs=4,
                                     name=f"tk{k}_{cb}")
                        x = xps[cb]
                        r2 = lambda lo: x[:, lo:lo + 32].rearrange(
                            "p (i two) c -> p i two c", two=2)
                        ra = r2(0)[:, :, 0]    # rows 0,2..30
                        rb = r2(1)[:, :, 0]    # rows 1,3..31
                        rc = r2(2)[:, :, 0]    # rows 2,4..32
                        rd = r2(2)[:, :, 1]    # rows 3,5..33
                        if k == 0:
                            eng.tensor_sub(tk[:], ra, rc)
                        elif k == 1:
                            eng.tensor_add(tk[:], rb, rc)
                        elif k == 2:
                            eng.tensor_sub(tk[:], rc, rb)
                        else:
                            eng.tensor_sub(tk[:], rb, rd)
                        tks.append(tk)
                    for c in range(4):
                        for cb in range(4):
                            eng = nc.gpsimd if cb == 3 else nc.vector
                            t = tks[cb]
                            c2 = lambda lo: t[:, :, lo:lo + 32].rearrange(
                                "p i (j two) -> p i j two", two=2)
                            ca = c2(0)[:, :, :, 0]
                            cbv = c2(1)[:, :, :, 0]
                            cc = c2(2)[:, :, :, 0]
                            cd = c2(2)[:, :, :, 1]
                            dst = vt[cb][:, 4 * k + c]
                            dst = dst.rearrange("p (i j) -> p i j", i=16)
                            if c == 0:
                                eng.tensor_sub(dst, ca, cc)
                            elif c == 1:
                                eng.tensor_add(dst, cbv, cc)
                            elif c == 2:
                                eng.tensor_sub(dst, cc, cbv)
                            else:
                                eng.tensor_sub(dst, cbv, cd)

            def emit_warm_ag():
                """Prime the collective path (TOPSP wake + descriptor
                staging) with an AllGather of the same shape as the real
                one, overlapped with the main compute."""
                warm_in = dram.tile([128, 16], f32, name="warm_in")
                warm_out = dram.tile([1024, 16], f32, addr_space="Shared",
                                     name="warm_out")
                warm_sb = sb.tile([128, 16], f32, tag="warm_sb")
                nc.vector.memset(warm_sb[:], 0.0)
                nc.scalar.dma_start(warm_in[:], warm_sb[:])
                nc.gpsimd.collective_compute(
                    "AllGather", ALU.bypass,
                    ins=[warm_in[:].opt()], outs=[warm_out[:].opt()],
                    replica_groups=[list(range(NCORES))])

            def wino_gen(cob, wuts=None):
                """Winograd conv block: 64 matmuls of [128,128]x[128,256]
                accumulating M_p over cin. Position pairs (2q, 2q+1) share
                one [128,512] PSUM bank (half each) and are evacuated with
                a single scalar copy, then the inverse transform
                Y = A^T M A runs on the vector engine."""
                if wuts is None:
                    wuts = [tp.tile([128, 4, 4, 128], bf, tag="wu", bufs=6,
                                    name=f"wu{cob}_{c}") for c in range(4)]
                    for c in range(4):
                        nc.sync.dma_start(wuts[c][:], wu[cob, c])
                raw = tp.tile([128, 1024], bf, tag="raw", bufs=4,
                              name=f"raw{cob}")
                msb = tp.tile([128, 16, 256], bf, tag="msb", bufs=2,
                              name=f"msb{cob}")
                yield raw
                for q in range(8):
                    mm = ps.tile([128, 512], f32, tag="wps", bufs=2,
                                 name=f"mm{cob}_{q}")
                    for sub in range(2):
                        p = 2 * q + sub
                        for cb in range(4):
                            nc.tensor.matmul(mm[:, 256 * sub:256 * (sub + 1)],
                                             wuts[p // 4][:, p % 4, cb],
                                             vt[cb][:, p],
                                             start=(cb == 0), stop=(cb == 3))
                    nc.scalar.copy(msb[:, 2 * q:2 * q + 2], mm[:])
                    yield None
                # inverse transform: stage1 P/Q, stage2 -> raw pixel classes
                pqt_ = tp.tile([128, 8, 256], bf, tag="pqt", bufs=2,
                               name=f"pqt{cob}")
                tw = tp.tile([128, 4, 256], bf, tag="wtmp", bufs=2,
                             name=f"tw{cob}")
                nc.vector.tensor_add(tw[:], msb[:, 0:4], msb[:, 4:8])
                nc.vector.tensor_add(pqt_[:, 0:4], tw[:], msb[:, 8:12])
                nc.vector.tensor_sub(tw[:], msb[:, 4:8], msb[:, 8:12])
                nc.vector.tensor_sub(pqt_[:, 4:8], tw[:], msb[:, 12:16])
                yield None
                s = pqt_[:].rearrange("p (g k) (i j) -> p g k i j",
                                      g=2, i=16)
                # raw is row-major over pixels: n = 64i + 32x + 2j + y, and
                # the P/Q row (g) of stage1 is the output-row parity x.
                rv = raw.rearrange("p (i x j y) -> p x y i j",
                                   i=16, x=2, j=16, y=2)
                tw2 = tp.tile([128, 2, 256], bf, tag="wtmp2", bufs=2,
                              name=f"tw2{cob}")
                t2v = tw2[:].rearrange("p g (i j) -> p g i j", i=16)
                nc.vector.tensor_add(t2v, s[:, :, 0], s[:, :, 1])
                nc.vector.tensor_add(rv[:, :, 0], t2v, s[:, :, 2])
                nc.vector.tensor_sub(t2v, s[:, :, 1], s[:, :, 2])
                nc.vector.tensor_sub(rv[:, :, 1], t2v, s[:, :, 3])
                yield None

            def post_gen(cob, raw):
                """Per-kind epilogue consuming a conv block's raw output."""
                if cob >= 8:   # v block: PE-transpose into vT
                    m = cob - 8
                    for c2 in range(2):
                        pt = ps.tile([128, 512], bf, tag="patt", bufs=4,
                                     name=f"pt{cob}_{c2}")
                        for c in range(4):
                            j = 4 * c2 + c
                            nc.tensor.transpose(pt[:, 128 * c:128 * (c + 1)],
                                                raw[:, 128 * j:128 * (j + 1)],
                                                identt[:])
                        dstv = vT[:, 4 * c2:4 * (c2 + 1), 128 * m:128 * (m + 1)]
                        srcv = pt[:].rearrange("p (a b) -> p a b", a=4)
                        if c2 == 0:
                            nc.scalar.copy(dstv, srcv)
                        else:
                            nc.vector.tensor_copy(out=dstv, in_=srcv)
                        yield None
                else:          # q/k block: cosine norms + normalized copy
                    m = cob % 4
                    dst = qhat if cob < 4 else khat
                    nrm = tp.tile([2, 1024], f32, tag="nrm", bufs=2,
                                  name=f"nrm{cob}")
                    inv = tp.tile([2, 1024], f32, tag="inv", bufs=2,
                                  name=f"inv{cob}")
                    # invb in bf16: the pbc broadcast matmul must not be
                    # fp32 -- an fp32 matmul runs as a LOW/HIGH double pass
                    # at ~1.9us each vs ~0.3us for bf16.
                    invb = tp.tile([2, 1024], bf, tag="invb", bufs=2,
                                   name=f"invb{cob}")
                    sq = tp.tile([128, 1024], bf, tag="sq", bufs=2,
                                 name=f"sq{cob}")
                    nc.gpsimd.tensor_mul(sq[:, 0:512], raw[:, 0:512],
                                         raw[:, 0:512])
                    nc.gpsimd.tensor_mul(sq[:, 512:1024], raw[:, 512:1024],
                                         raw[:, 512:1024])
                    yield None
                    for t in range(2):
                        pss = ps.tile([2, 512], f32, tag="patt", bufs=4,
                                      name=f"pss{cob}_{t}")
                        nc.tensor.matmul(pss[:], ones2t[:],
                                         sq[:, 512 * t:512 * (t + 1)],
                                         start=True, stop=True)
                        nc.scalar.activation(out=nrm[:, 512 * t:512 * (t + 1)],
                                             in_=pss[:], func=AF.Sqrt,
                                             bias=smt[:], scale=1.0)
                        yield None
                    nc.vector.reciprocal_approx_fast(out=inv[:], in_=nrm[:])
                    nc.vector.tensor_copy(out=invb[:], in_=inv[:])
                    yield None
                    for t in range(2):
                        pbc = ps.tile([128, 512], f32, tag="patt", bufs=4,
                                      name=f"pbc{cob}_{t}")
                        nc.tensor.matmul(pbc[:], sel2t[:],
                                         invb[:, 512 * t:512 * (t + 1)],
                                         start=True, stop=True)
                        nc.vector.tensor_mul(dst[:, m, 512 * t:512 * (t + 1)],
                                             raw[:, 512 * t:512 * (t + 1)],
                                             pbc[:])
                        yield None

            dumc = [0]

            def emit_duty(n=2):
                """Full-array N=256 matmuls into a scratch PSUM tile. The
                attention stream's own matmuls are 64x64 quadrant tiles,
                which read as low activity to the HAM monitor; once the
                Winograd cover stream ends, the monitor would halve the PE
                clock for the whole attention tail. ~0.2us of dense filler
                per ~1us chunk keeps the activity window above the throttle
                threshold."""
                dm = ps.tile([128, 256], f32, tag="wps", bufs=2,
                             name=f"dm{dumc[0]}")
                dumc[0] += 1
                for i in range(n):
                    nc.tensor.matmul(dm[:], identt[:], vt[i % 4][:, 0],
                                     start=(i == 0), stop=(i == n - 1))

            def att_gen(m):
                """Attention pair (heads 2m, 2m+1). The qk product packs
                all four 64x64 PE quadrants: rows split by head (d=64 each),
                columns split by j-pixel 64-blocks, so the four matmuls of
                one i-half run concurrently."""
                pot = [ps.tile([128, 512], f32, tag="pacc", bufs=2,
                               name=f"po{m}_{t}") for t in range(2)]
                prev = None
                for sj in range(8):
                    if prev is not None:
                        emit_outT(m, pot, *prev)
                    # the last two pairs run mostly after the Winograd
                    # cover stream has ended; denser filler holds the HAM
                    # window above the throttle threshold there.
                    emit_duty(3 if m >= 2 else 2)
                    pa = [[ps.tile([128, 512], f32, tag="patt", bufs=4,
                                   name=f"pa{m}_{sj}_{h}_{t}")
                           for t in range(2)] for h in range(2)]
                    for t in range(2):
                        for h in range(2):
                            for jb in range(2):
                                nc.tensor.matmul(
                                    pa[h][t][64 * jb:64 * (jb + 1), :],
                                    khat[64 * h:64 * (h + 1), m,
                                         128 * sj + 64 * jb:
                                         128 * sj + 64 * (jb + 1)],
                                    qhat[64 * h:64 * (h + 1), m,
                                         512 * t:512 * (t + 1)],
                                    start=True, stop=True)
                    a0 = tp.tile([128, 1024], bf, tag="attnT", bufs=6,
                                 name=f"a0_{m}_{sj}")
                    a1 = tp.tile([128, 1024], bf, tag="attnT", bufs=6,
                                 name=f"a1_{m}_{sj}")
                    nc.scalar.copy(a0[:, 0:512], pa[0][0][:])
                    nc.vector.tensor_copy(out=a0[:, 512:1024], in_=pa[0][1][:])
                    nc.scalar.copy(a1[:, 0:512], pa[1][0][:])
                    nc.vector.tensor_copy(out=a1[:, 512:1024], in_=pa[1][1][:])
                    prev = (sj, a0, a1)
                    yield None
                emit_outT(m, pot, *prev)
                emit_duty()
                for t in range(2):
                    sl = slice(512 * t, 512 * (t + 1))
                    if t == 0:
                        nc.scalar.copy(att[:, m, sl], pot[t][:])
                    else:
                        nc.vector.tensor_copy(out=att[:, m, sl], in_=pot[t][:])
                yield None

            def emit_outT(m, pot, j, a0, a1):
                for t in range(2):
                    nc.tensor.matmul(pot[t][0:64, :],
                                     vT[:, j, 128 * m:128 * m + 64],
                                     a0[:, 512 * t:512 * (t + 1)],
                                     start=(j == 0), stop=(j == 7),
                                     tile_position=(0, 0))
                    nc.tensor.matmul(pot[t][64:128, :],
                                     vT[:, j, 128 * m + 64:128 * (m + 1)],
                                     a1[:, 512 * t:512 * (t + 1)],
                                     start=(j == 0), stop=(j == 7),
                                     tile_position=(0, 64))

            def conv1x1_gen():
                for c4 in range(4):
                    pys = []
                    for t in range(2):
                        py = ps.tile([128, 512], f32, tag="pacc", bufs=2,
                                     name=f"py{c4}_{t}")
                        for cb in range(4):
                            nc.tensor.matmul(py[:],
                                             wot[:, cb, 128 * c4:128 * (c4 + 1)],
                                             att[:, cb, 512 * t:512 * (t + 1)],
                                             start=(cb == 0), stop=(cb == 3))
                        pys.append(py)
                    yield None
                    ytv = yt[:, c4, :].rearrange("p (t f) -> p t f", t=2)
                    nc.vector.tensor_scalar(
                        out=ytv[:, 0], in0=pys[0][:],
                        scalar1=1.0, scalar2=None,
                        op0=ALU.mult, op1=ALU.add,
                        accum_out=part[:, 2 * c4:2 * c4 + 1])
                    bscr = tp.tile([128, 1024], bf, tag="bscr", bufs=2,
                                   name=f"bscr{c4}")
                    nc.scalar.activation(out=bscr[:, 0:512], in_=pys[0][:],
                                         func=AF.Square,
                                         accum_out=part[:, 8 + 2 * c4:
                                                        9 + 2 * c4])
                    yield None
                    nc.vector.tensor_scalar(
                        out=ytv[:, 1], in0=pys[1][:],
                        scalar1=1.0, scalar2=None,
                        op0=ALU.mult, op1=ALU.add,
                        accum_out=part[:, 2 * c4 + 1:2 * c4 + 2])
                    nc.scalar.activation(out=bscr[:, 512:1024], in_=pys[1][:],
                                         func=AF.Square,
                                         accum_out=part[:, 9 + 2 * c4:
                                                        10 + 2 * c4])
                    yield None

            def drain(g):
                if g is not None:
                    for _ in g:
                        pass

            def chain(*gens):
                for g in gens:
                    yield from g

            def delay_gen(n):
                """Emission-delay tokens: each yield lets one rotation pull
                pass without emitting, so a chain spliced right after a
                producer post can't race ahead of the post's PE-queue
                emissions (which would deadlock the FIFO tensor queue)."""
                for _ in range(n):
                    yield None

            # ---- emission plan ----
            # V transform first (vector+gpsimd queues fill while input DMAs
            # land), then the 12 Winograd blocks [v8..v11, q0,k4, q1,k5,
            # q2,k6, q3,k7] with epilogues woven in via the filler rotation.
            # The attention pairs run as ONE serial chain that starts right
            # after k4's post has drained; the dense Winograd matmul stream
            # covers the attention evacuation copies. The k5/k6/k7 posts are
            # spliced INTO the chain before the att pair that reads them:
            # the tensor queue is FIFO, so a qk matmul emitted ahead of its
            # producer's bcast matmul would deadlock the queue.
            emit_v_transform()

            # wot/gbt are only needed by conv1x1/BN at the end; issue their
            # DMAs behind the gpsimd transform ops so the startup queues
            # stay clear.
            for cb in range(4):
                nc.gpsimd.dma_start(wot[:, cb], wo[cb])
            nc.gpsimd.dma_start(gbt[:], gb)

            fillers = []

            def pull_filler():
                while fillers:
                    g = fillers[0]
                    try:
                        next(g)
                        fillers.append(fillers.pop(0))
                        return
                    except StopIteration:
                        fillers.pop(0)

            # Blocks 8/9 interleave at half-block granularity: the V
            # transform produces vt[:, p] at about half the rate one block
            # consumes it, so alternating two consumers halves the startup
            # stall; their weight DMAs interleave chunk-wise on the sync
            # queue so neither starves.
            wp8 = [tp.tile([128, 4, 4, 128], bf, tag="wu", bufs=6,
                           name=f"wu8_{c}") for c in range(4)]
            wp9 = [tp.tile([128, 4, 4, 128], bf, tag="wu", bufs=6,
                           name=f"wu9_{c}") for c in range(4)]
            for c in range(4):
                nc.sync.dma_start(wp8[c][:], wu[8, c])
                nc.sync.dma_start(wp9[c][:], wu[9, c])
            g8 = wino_gen(8, wp8)
            g9 = wino_gen(9, wp9)
            raw8 = next(g8)
            raw9 = next(g9)
            for i in range(4):
                next(g8)
                emit_duty_early()
            for i in range(4):
                next(g9)
                emit_duty_early()
            for i in range(6):
                next(g8, None)
                if i % 2 == 0:
                    emit_duty_early()
            for i in range(6):
                next(g9, None)
                if i % 2 == 0:
                    emit_duty_early()
            fillers.append(post_gen(8, raw8))
            fillers.append(post_gen(9, raw9))
            emit_warm_ag()

            # q/k blocks alternate (q0,k4, q1,k5, ...) so the attention
            # chain's dependencies complete as early as possible and the
            # remaining Winograd blocks cover more of the attention stream.
            for cob in [10, 11, 0, 4, 1, 5, 2, 6, 3, 7]:
                g = wino_gen(cob)
                raw = next(g)
                first = True
                for _ in g:
                    if not first:
                        pull_filler()
                    first = False
                fillers.append(post_gen(cob, raw))
                # att0 splices right after its producers (q0, k4) with a
                # double pull share so it runs fully under the Winograd
                # cover; the later pairs splice after k5 with delay tokens
                # sized so each att_m's emission trails its khat producer's.
                if cob == 4:
                    ch0 = chain(delay_gen(8), att_gen(0))
                    fillers.append(ch0)
                    fillers.append(ch0)
                if cob == 5:
                    fillers.append(chain(delay_gen(8), att_gen(1),
                                         delay_gen(8), att_gen(2),
                                         delay_gen(6), att_gen(3)))
            while fillers:
                pull_filler()
            drain(conv1x1_gen())

            # ---- BatchNorm: AllGather 8KB of partial sums, local tree
            # reduce, then apply. ----
            cin_d = dram.tile([128, 16], f32)
            cgat_d = dram.tile([1024, 16], f32, addr_space="Shared")
            nc.sync.dma_start(cin_d[:], part[:])
            nc.gpsimd.collective_compute(
                "AllGather", ALU.bypass,
                ins=[cin_d[:].opt()], outs=[cgat_d[:].opt()],
                replica_groups=[list(range(NCORES))])
            gtv = cgat_d.rearrange("(r p) f -> p r f", r=8)
            nc.sync.dma_start(gt[:, 0:4], gtv[:, 0:4])
            nc.scalar.dma_start(gt[:, 4:8], gtv[:, 4:8])

            var = sb.tile([128, 4], f32, tag="var")
            stdt = sb.tile([128, 4], f32, tag="stdt")
            rstd = sb.tile([128, 4], f32, tag="rstd")
            scl = sb.tile([128, 4], f32, tag="scl")
            sht = sb.tile([128, 4], f32, tag="sht")
            msq = sb.tile([128, 4], f32, tag="msq")
            tmp = sb.tile([128, 4], f32, tag="tmp")
            comb = sb.tile([128, 8], f32, tag="comb")
            NINV = 1.0 / 8192.0
            nc.vector.tensor_add(t4[:], gt[:, 0:4], gt[:, 4:8])
            nc.vector.tensor_add(t2[:], t4[:, 0:2], t4[:, 2:4])
            nc.vector.tensor_add(stats[:], t2[:, 0], t2[:, 1])
            nc.vector.tensor_scalar_mul(stats[:], stats[:], NINV)
            pairv = stats[:].rearrange("p (a c two) -> p a c two",
                                       a=2, two=2)
            combv = comb[:].rearrange("p (a c) -> p a c", a=2)
            nc.vector.tensor_add(combv, pairv[:, :, :, 0], pairv[:, :, :, 1])
            mean = comb[:, 0:4]
            ex2 = comb[:, 4:8]
            nc.vector.tensor_mul(msq[:], mean[:], mean[:])
            nc.vector.tensor_sub(var[:], ex2[:], msq[:])
            nc.scalar.activation(out=stdt[:], in_=var[:], func=AF.Sqrt,
                                 bias=epst[:], scale=1.0)
            nc.vector.reciprocal_approx_fast(out=rstd[:], in_=stdt[:])
            nc.vector.tensor_mul(scl[:], gbt[:, 0:4], rstd[:])
            nc.vector.tensor_mul(tmp[:], mean[:], scl[:])
            nc.vector.tensor_sub(sht[:], gbt[:, 4:8], tmp[:])

            # BN apply + ReLU on 512-column halves split scalar/vector, each
            # half DMA'd out (un-permuting the tiled pixel order) as soon as
            # its engine finishes.
            dma_q = [nc.sync, nc.gpsimd, nc.scalar, nc.sync]
            qi = 0
            for c4 in range(4):
                for h in range(2):
                    sl = slice(512 * h, 512 * (h + 1))
                    seg = yt[:, c4, sl]
                    if h == 0:
                        nc.scalar.activation(out=seg, in_=seg, func=AF.Relu,
                                             scale=scl[:, c4:c4 + 1],
                                             bias=sht[:, c4:c4 + 1])
                    else:
                        nc.vector.tensor_scalar(out=seg, in0=seg,
                                                scalar1=scl[:, c4:c4 + 1],
                                                scalar2=sht[:, c4:c4 + 1],
                                                op0=ALU.mult, op1=ALU.add)
                        nc.vector.tensor_scalar_max(out=seg, in0=seg,
                                                    scalar1=0.0)
                    dma_q[qi % 4].dma_start(
                        out[128 * c4:128 * (c4 + 1), sl], seg)
                    qi += 1

    nc.compile()
    return nc


def _prep_inputs(x, W_qkv, W_out, gamma, beta):
    x = np.asarray(x, np.float32)
    W_qkv = np.asarray(W_qkv, np.float32)
    W_out = np.asarray(W_out, np.float32)
    gamma = np.asarray(gamma, np.float32)
    beta = np.asarray(beta, np.float32)

    xs = x.reshape(8, 4, 128, 32, 32)
    xpad = np.zeros((8, 4, 128, 34, 34), np.float32)
    xpad[:, :, :, 1:33, 1:33] = xs
    xpad = xpad.astype(BF)

    G = np.array([[1, 0, 0], [.5, .5, .5], [.5, -.5, .5], [0, 0, 1]],
                 np.float32)
    U4 = np.einsum('ru,oiuv,cv->rcoi', G, W_qkv, G)     # [4,4,1536,512]
    t = U4.reshape(4, 4, 12, 128, 4, 128)               # r c cob co cb ci
    wu = np.ascontiguousarray(
        t.transpose(2, 0, 5, 1, 4, 3).astype(BF))       # cob r ci c cb co

    wo = np.ascontiguousarray(
        W_out[:, :, 0, 0].T.reshape(4, 128, 512).astype(BF))
    gb = np.ascontiguousarray(np.concatenate(
        [gamma.reshape(4, 128).T, beta.reshape(4, 128).T], axis=1)
        .astype(np.float32))
    p = np.arange(128)
    ones2 = np.ascontiguousarray(
        np.stack([p < 64, p >= 64], axis=1).astype(BF))
    sel2 = np.ascontiguousarray(
        np.stack([p < 64, p >= 64], axis=0).astype(BF))
    identv = np.eye(128, dtype=BF)

    common = {"wu": wu, "wo": wo, "gb": gb,
              "ones2": ones2, "sel2": sel2, "ident": identv}
    return [{"xpad": np.ascontiguousarray(xpad[b]), **common}
            for b in range(8)]


def kernel(x, W_qkv, W_out, gamma, beta):
    global _NC, LAST_RESULT
    if _NC is None:
        _NC = _build()
    in_maps = _prep_inputs(x, W_qkv, W_out, gamma, beta)
    res = bass_utils.run_bass_kernel_spmd(
        _NC, in_maps, core_ids=list(range(NCORES)))
    LAST_RESULT = res
    outs = [res.results[b]["out"].reshape(512, 32, 32) for b in range(8)]
    return np.stack(outs).astype(np.float32)
